# revision 26
# baseline (speedup 1.0000x reference)
"""GumbelSelector Trainium2 kernel.

Math: h = relu(s @ W1 + b1); lo = h @ W2 + b2  (2 classes)
  dec  = (argmax(lo) == 1)  ==  (z > 0)         where z = h @ (W2[:,1]-W2[:,0]) + (b2[1]-b2[0])
  prob = softmax(lo)[..., 1] ==  sigmoid(z)     and dec == (prob > 0.5)
  Per-row correction (LB=1): if a row of dec is all zero, activate argmax(rnoise).

Sharding: data-parallel over batch B=64 -> 8 cores x 8 rows. Weights replicated.
Host pre-arranges each core's s shard slab-major ([slab, 128, 2*SLAB]) so every
slab is ONE fully-coalesced 2 MiB DMA with the contraction dim on partitions.

Matmuls run as float32r (full fp32 bits in memory, PE rounds internally):
1 cycle/row instead of fp32's multi-pass. That moves the bottleneck from the
tensor engine to a near-balanced split across PE (3 column passes/token),
DVE (relu + dec/correction), ACT (sigmoid strips), and the HBM stream.
"""

import sys

if "/opt/trn_rl_repo" not in sys.path:
    sys.path.insert(0, "/opt/trn_rl_repo")

import numpy as np

import concourse.bass as bass
import concourse.mybir as mybir
import concourse.tile as tile
from concourse import bacc
from concourse.bass_utils import run_bass_kernel_spmd

B, N, D = 64, 4096, 256
HID = D // 2  # 128
NCORES = 8
BPC = B // NCORES          # batch rows per core
TOK = BPC * N              # 32768 tokens per core
SLAB = 2048                # tokens per DMA slab (one contiguous 2 MiB load)
NSLAB = TOK // SLAB
TS = 1024                  # tokens per compute tile (2 PSUM banks)
F32 = mybir.dt.float32
F32R = mybir.dt.float32r

_NC = None


def _build_nc(reps=1, mmdt=F32R, variant="full"):
    nc = bacc.Bacc("TRN2", target_bir_lowering=False, debug=False)
    # matmul operands carry mmdt (float32 or float32r) end to end: the BIR
    # verifier requires every producer feeding an fp32r matmul to be marked
    # as rounded, so the DRAM tensors and SBUF tiles are declared mmdt
    sS = nc.dram_tensor("sS", [NSLAB * 128, 2 * SLAB], mmdt, kind="ExternalInput")
    rn = nc.dram_tensor("rn", [BPC, N], F32, kind="ExternalInput")
    w1 = nc.dram_tensor("w1", [D, HID], mmdt, kind="ExternalInput")
    b1 = nc.dram_tensor("b1", [HID, 1], F32, kind="ExternalInput")
    w2d = nc.dram_tensor("w2d", [HID, 1], mmdt, kind="ExternalInput")
    b2d = nc.dram_tensor("b2d", [1, 1], F32, kind="ExternalInput")
    dec = nc.dram_tensor("dec", [BPC, N], F32, kind="ExternalOutput")
    prob = nc.dram_tensor("prob", [BPC, N], F32, kind="ExternalOutput")

    AF = mybir.ActivationFunctionType
    ALU = mybir.AluOpType

    with tile.TileContext(nc) as tc:
        with (
            tc.tile_pool(name="consts", bufs=1) as consts,
            tc.tile_pool(name="io8", bufs=1) as io8,
            tc.tile_pool(name="sload", bufs=3) as sload,
            tc.tile_pool(name="hpool", bufs=3) as hpool,
            tc.tile_pool(name="cpool", bufs=2) as cpool,
            tc.tile_pool(name="phpool", bufs=2, space=bass.MemorySpace.PSUM) as phpool,
            tc.tile_pool(name="pzpool", bufs=2, space=bass.MemorySpace.PSUM) as pzpool,
        ):
            w1a = consts.tile([128, HID], mmdt)
            nc.sync.dma_start(w1a[:], w1[0:128, :])
            w1b = consts.tile([128, HID], mmdt)
            nc.sync.dma_start(w1b[:], w1[128:256, :])
            b1s = consts.tile([HID, 1], F32)
            nc.sync.dma_start(b1s[:], b1[:])
            w2s = consts.tile([HID, 1], mmdt)
            nc.sync.dma_start(w2s[:], w2d[:])
            b2s = consts.tile([1, 1], F32)
            nc.sync.dma_start(b2s[:], b2d[:])
            rns = io8.tile([BPC, N], F32)
            nc.sync.dma_start(rns[:], rn[:])

            # engines may only address base partition 0/32/64/96, so prob
            # chunks are computed on partition 0 into a per-row accumulator;
            # row DMAs collect them into prob8 and dec is derived from prob
            dec8 = io8.tile([BPC, N], F32)
            prob8 = io8.tile([BPC, N], F32)
            rmaxr = io8.tile([BPC, 1], F32)
            nc.vector.tensor_reduce(rmaxr[:], rns[:], mybir.AxisListType.X, ALU.max)

            for rep in range(reps):
                if variant == "dmaonly":
                    acc = io8.tile([128, 1], F32, tag="acc")
                    for si in range(NSLAB):
                        st = sload.tile([128, 2 * SLAB], mmdt, tag="st")
                        nc.sync.dma_start(st[:], sS[si * 128 : (si + 1) * 128, :])
                        # cheap consumer so the measurement stays DMA-bound
                        nc.vector.tensor_reduce(
                            acc[:], st[:, 0:64].bitcast(F32),
                            mybir.AxisListType.X, ALU.max,
                        )
                    nc.sync.dma_start(dec[0:1, 0:128], acc[:].transpose([1, 0]))
                    nc.sync.dma_start(prob[0:1, 0:128], acc[:].transpose([1, 0]))
                    continue

                st = None
                for b_row in range(BPC):
                    pcrow = cpool.tile([1, N], F32, tag="pcrow")
                    for ci in range(N // TS):
                        col = ci * TS
                        toff = b_row * N + col
                        si, hoff = toff // SLAB, toff % SLAB
                        if hoff == 0:
                            st = sload.tile([128, 2 * SLAB], mmdt, tag="st")
                            nc.sync.dma_start(
                                st[:], sS[si * 128 : (si + 1) * 128, :]
                            )
                        ph = phpool.tile([128, TS], F32)
                        # same stationary back to back to minimize LDWEIGHTS swaps
                        nc.tensor.matmul(ph[:, 0:512], w1a[:],
                                         st[:, hoff : hoff + 512],
                                         start=True, stop=False)
                        nc.tensor.matmul(ph[:, 512:1024], w1a[:],
                                         st[:, hoff + 512 : hoff + 1024],
                                         start=True, stop=False)
                        nc.tensor.matmul(ph[:, 0:512], w1b[:],
                                         st[:, SLAB + hoff : SLAB + hoff + 512],
                                         start=False, stop=True)
                        nc.tensor.matmul(ph[:, 512:1024], w1b[:],
                                         st[:, SLAB + hoff + 512 : SLAB + hoff + 1024],
                                         start=False, stop=True)
                        if variant == "mmonly":
                            continue
                        # relu(x + b1) on DVE; ACT is saturated by the sigmoids
                        h = hpool.tile([128, TS], mmdt)
                        if variant == "fullact":
                            nc.scalar.activation(h[:], ph[:], AF.Relu, bias=b1s[:])
                        else:
                            nc.vector.tensor_scalar(h[:], ph[:], b1s[:], 0.0,
                                                    ALU.add, ALU.max)
                        if variant == "mmrelu":
                            continue
                        pz = pzpool.tile([1, TS], F32)
                        nc.tensor.matmul(pz[0:1, 0:512], w2s[:],
                                         h[:, 0:512],
                                         start=True, stop=True)
                        nc.tensor.matmul(pz[0:1, 512:1024], w2s[:],
                                         h[:, 512:1024],
                                         start=True, stop=True)
                        if variant == "mml2":
                            continue
                        nc.scalar.activation(pcrow[0:1, col : col + TS],
                                             pz[0:1, :], AF.Sigmoid, bias=b2s[:])
                    if variant in ("mmonly", "mmrelu", "mml2"):
                        continue
                    nc.sync.dma_start(prob8[b_row : b_row + 1, :], pcrow[:])

                if variant == "full":
                    # dec = prob > 0.5 (sigmoid monotone; z>0 <=> prob>0.5)
                    nc.vector.tensor_scalar(dec8[:], prob8[:], 0.5, None, ALU.is_gt)
                    rmaxd = io8.tile([BPC, 1], F32)
                    nc.vector.tensor_reduce(rmaxd[:], dec8[:],
                                            mybir.AxisListType.X, ALU.max)
                    need = io8.tile([BPC, 1], F32)
                    nc.vector.tensor_scalar(need[:], rmaxd[:], 0.0, None,
                                            ALU.is_equal)
                    # rows needing correction activate argmax(rnoise)
                    fix = io8.tile([BPC, N], F32)
                    nc.vector.tensor_scalar(fix[:], rns[:], rmaxr[:], need[:],
                                            ALU.is_equal, ALU.mult)
                    decf = io8.tile([BPC, N], F32)
                    nc.vector.tensor_max(decf[:], dec8[:], fix[:])
                    nc.sync.dma_start(dec[:], decf[:])
                if variant in ("full", "fullact"):
                    nc.sync.dma_start(prob[:], prob8[:])

    nc.compile()
    return nc


def _get_nc():
    global _NC
    if _NC is None:
        _NC = _build_nc()
    return _NC


def _make_in_maps(s, W1, b1, W2, b2, rnoise):
    s = np.ascontiguousarray(s, dtype=np.float32)
    w1 = np.ascontiguousarray(W1, dtype=np.float32)
    b1c = np.ascontiguousarray(b1, dtype=np.float32).reshape(HID, 1)
    w2dc = np.ascontiguousarray(W2[:, 1] - W2[:, 0], dtype=np.float32).reshape(HID, 1)
    b2dv = np.float32(b2[1] - b2[0])
    b2dc = np.array([[b2dv]], dtype=np.float32)
    rn = np.ascontiguousarray(rnoise, dtype=np.float32)

    # slab-major: [core][slab, partition, khalf*SLAB + token] so each slab is
    # one fully-contiguous 2 MiB DMA with the contraction dim on partitions
    sS = np.ascontiguousarray(
        s.reshape(NCORES, NSLAB, SLAB, 2, 128).transpose(0, 1, 4, 3, 2)
    ).reshape(NCORES, NSLAB * 128, 2 * SLAB)
    return [
        {
            "sS": sS[c],
            "rn": rn.reshape(NCORES, BPC, N)[c],
            "w1": w1,
            "b1": b1c,
            "w2d": w2dc,
            "b2d": b2dc,
        }
        for c in range(NCORES)
    ]


def _assemble(results):
    dec = np.concatenate(
        [r["dec"].reshape(BPC, N) for r in results], axis=0
    )
    prob = np.concatenate(
        [r["prob"].reshape(BPC, N) for r in results], axis=0
    )
    return dec, prob


def run(s, W1, b1, W2, b2, rnoise, trace=False):
    nc = _get_nc()
    in_maps = _make_in_maps(s, W1, b1, W2, b2, rnoise)
    res = run_bass_kernel_spmd(nc, in_maps, list(range(NCORES)), trace=trace)
    return _assemble(res.results), res


def kernel(s, W1, b1, W2, b2, rnoise):
    (dec, prob), _ = run(s, W1, b1, W2, b2, rnoise)
    return dec, prob
